# revision 2
# baseline (speedup 1.0000x reference)
"""Cross-attention kernel for Trainium2, 8 NeuronCores — v3.

Problem (hardcoded): B=4, SQ=SK=2048, DIM=1024, fp32 in/out.
    q = x1 @ Wq^T + bq ; k = x2 @ Wk^T + bk ; v = x2 @ Wv^T + bv
    out = softmax(q k^T / sqrt(D)) v

Sharding: core c = 2b+h handles batch b, query rows [1024h, 1024h+1024).
K/V projections are j-split across the pair; halves are exchanged with
FOUR pairwise 1MB AllGathers (ktA, ktB, vA, vB) so each CC triggers as
soon as its quarter is projected, and the ~60 GB/s CC wire time hides
under Q-proj + scores. The scores/PV key-tile order follows CC arrival
order: global jt {0-3, 8-11} (A) first, then {4-7, 12-15} (B).

PE order: K -> Q -> V -> scores(ih0) -> scores(ih1) -> PV(ih0) -> PV(ih1),
maximizing the distance from each CC trigger to its first consumer.

All matmuls are bf16 (1 PE cycle/row, half the DMA/SBUF of f32),
accumulating fp32 in PSUM. Softmax skips max-subtraction (scores are
O(1)). Scores are computed transposed S^T[j,i] and output as out^T[e,i]
so no transposes are needed; PV accumulates all 16 key tiles of a query
half into a single PSUM bank. Softmax denominators come from ones-row
matmuls lagged one tile behind exp so the PE never waits; bv is added at
the end (sum_j p_j = 1). The host transposes [e,i] back.
"""

import os
import numpy as np
import ml_dtypes

import concourse.bass as bass
import concourse.tile as tile
from concourse import bacc, mybir
from concourse.bass_utils import run_bass_kernel_spmd

B, SQ, SK, D = 4, 2048, 2048, 1024
N_CORES = 8
QH = SQ // 2  # queries per core
KH = SK // 2  # keys projected per core
SCALE = 1.0 / np.sqrt(D)

F32 = mybir.dt.float32
F32R = mybir.dt.float32r
BF16 = mybir.dt.bfloat16
NPBF16 = ml_dtypes.bfloat16

DT = D // 128
ET = D // 128
JT_G = SK // 128  # 16 global key tiles
JT_L = KH // 128  # 8 local key tiles
IH = QH // 512  # 2 query column halves

# key-tile order matching CC arrival: A = local j [0,512) of both ranks,
# B = local j [512,1024) of both ranks
JTO = [0, 1, 2, 3, 8, 9, 10, 11, 4, 5, 6, 7, 12, 13, 14, 15]

PAIRS = [[0, 1], [2, 3], [4, 5], [6, 7]]

_CACHE = {}

LAST_EXEC_NS = None
LAST_RESULTS = None


def _maybe_enable_trace():
    """Best-effort install of the NTFF profile hook (stripped axon client)."""
    try:
        import sys
        import types

        if "antenv.axon_hooks" not in sys.modules:
            mod = types.ModuleType("antenv.axon_hooks")
            _hook = [None]
            mod.set_axon_ntff_profile_hook = lambda h: _hook.__setitem__(0, h)
            mod.get_axon_ntff_profile_hook = lambda: _hook[0]
            import antenv

            antenv.axon_hooks = mod
            sys.modules["antenv.axon_hooks"] = mod
            from trn_agent_boot.trn_boot import _ntff_profile_via_ctypes

            mod.set_axon_ntff_profile_hook(
                _ntff_profile_via_ctypes("/opt/axon/libaxon_pjrt.so")
            )
            from concourse import bass_utils

            bass_utils.upload_artifacts = lambda tmpdir: f"local:{tmpdir}"
        return True
    except Exception:
        return False


def _build():
    nc = bacc.Bacc()

    x1hT = nc.dram_tensor("x1hT", [D, QH], BF16, kind="ExternalInput")
    x2hT = nc.dram_tensor("x2hT", [D, KH], BF16, kind="ExternalInput")
    WqT = nc.dram_tensor("WqT", [D, D], BF16, kind="ExternalInput")
    WkT = nc.dram_tensor("WkT", [D, D], BF16, kind="ExternalInput")
    WvT = nc.dram_tensor("WvT", [D, D], BF16, kind="ExternalInput")
    bqs = nc.dram_tensor("bqs", [128, 8], F32, kind="ExternalInput")
    bks = nc.dram_tensor("bks", [128, 8], F32, kind="ExternalInput")
    bvs = nc.dram_tensor("bvs", [128, 8], F32, kind="ExternalInput")
    onescb = nc.dram_tensor("onescb", [128, 1], BF16, kind="ExternalInput")
    onesr = nc.dram_tensor("onesr", [1, 128], F32R, kind="ExternalInput")
    outT = nc.dram_tensor("outT", [D, QH], F32, kind="ExternalOutput")

    x1r = x1hT.rearrange("(dt p) i -> p dt i", p=128)
    x2r = x2hT.rearrange("(dt p) j -> p dt j", p=128)
    wqr = WqT.rearrange("(dt p) e -> p dt e", p=128)
    wkr = WkT.rearrange("(dt p) e -> p dt e", p=128)
    wvr = WvT.rearrange("(dt p) e -> p dt e", p=128)

    IDENT = mybir.ActivationFunctionType.Identity
    EXP = mybir.ActivationFunctionType.Exp

    with tile.TileContext(nc) as tc:
        with (
            tc.tile_pool(name="dram", bufs=1, space="DRAM") as dram,
            tc.tile_pool(name="persist", bufs=1) as persist,
            tc.tile_pool(name="ps_proj", bufs=2, space="PSUM") as ps_proj,
            tc.tile_pool(name="ps_out", bufs=3, space="PSUM") as ps_out,
            tc.tile_pool(name="ps_l", bufs=2, space="PSUM") as ps_l,
            tc.tile_pool(name="ps_rlb", bufs=1, space="PSUM") as ps_rlb,
        ):
            # CC bounce buffers (Local Internal DRAM), quartered
            kt_bin = [
                dram.tile([128, ET, 512], BF16, tag=f"ktbin{a}", name=f"ktbin{a}")
                for a in range(2)
            ]
            kt_bout = [
                dram.tile([2, 128, ET, 512], BF16, tag=f"ktbout{a}", name=f"ktbout{a}")
                for a in range(2)
            ]
            v_bin = [
                dram.tile([128, 4, D], BF16, tag=f"vbin{a}", name=f"vbin{a}")
                for a in range(2)
            ]
            v_bout = [
                dram.tile([2, 128, 4, D], BF16, tag=f"vbout{a}", name=f"vbout{a}")
                for a in range(2)
            ]

            # persistent SBUF
            bq_sb = persist.tile([128, 8], F32, tag="bq")
            bk_sb = persist.tile([128, 8], F32, tag="bk")
            bvs_sb = persist.tile([128, 8], F32, tag="bvs")
            onescb_sb = persist.tile([128, 1], BF16, tag="onescb")
            onesr_sb = persist.tile([1, 128], F32R, tag="onesr")
            kt_sb = persist.tile([128, ET, SK], BF16, tag="kt")  # k^T [e, j]
            v_sb = persist.tile([128, JT_G, D], BF16, tag="v")  # v [j, e]
            qt_sb = persist.tile([128, ET, QH], BF16, tag="qt")  # q^T [e, i]
            # staging for own-half projections; persistent so the ph2 ex
            # tiles don't overlap them (pool reuse would make the first
            # exp wait on the v bounce DMA draining v_own)
            kt_own = persist.tile([128, ET, KH], BF16, tag="kto")
            v_own = persist.tile([128, JT_L, D], BF16, tag="vo")

            # small constants on the SWDGE ring (keeps HWDGE rings clear)
            nc.gpsimd.dma_start(out=bk_sb, in_=bks[:, :])
            nc.gpsimd.dma_start(out=bq_sb, in_=bqs[:, :])
            nc.gpsimd.dma_start(out=onescb_sb, in_=onescb[:, :])
            nc.gpsimd.dma_start(out=bvs_sb, in_=bvs[:, :])
            nc.gpsimd.dma_start(out=onesr_sb, in_=onesr[:, :])

            with tc.tile_pool(name="ph1", bufs=1) as ph1:
                wk_sb = ph1.tile([128, DT, D], BF16, tag="wk")
                wv_sb = ph1.tile([128, DT, D], BF16, tag="wv")
                wq_sb = ph1.tile([128, DT, D], BF16, tag="wq")
                x2_sb = ph1.tile([128, DT, KH], BF16, tag="x2")
                x1_sb = ph1.tile([128, DT, QH], BF16, tag="x1")

                # DMA issue order per ring == need order; first x2 chunk is
                # narrow so the first K-proj matmuls start sooner
                nc.sync.dma_start(out=wk_sb[:, :, 0:512], in_=wkr[:, :, 0:512])
                nc.scalar.dma_start(out=x2_sb[:, :, 0:256], in_=x2r[:, :, 0:256])
                nc.scalar.dma_start(out=x2_sb[:, :, 256:512], in_=x2r[:, :, 256:512])
                nc.sync.dma_start(out=wk_sb[:, :, 512:1024], in_=wkr[:, :, 512:1024])
                nc.scalar.dma_start(
                    out=x2_sb[:, :, 512:1024], in_=x2r[:, :, 512:1024]
                )
                nc.sync.dma_start(out=wq_sb[:, :, 0:512], in_=wqr[:, :, 0:512])
                nc.sync.dma_start(out=wq_sb[:, :, 512:1024], in_=wqr[:, :, 512:1024])
                nc.scalar.dma_start(out=x1_sb[:, :, 0:512], in_=x1r[:, :, 0:512])
                nc.scalar.dma_start(
                    out=x1_sb[:, :, 512:1024], in_=x1r[:, :, 512:1024]
                )
                nc.sync.dma_start(out=wv_sb[:, :, 0:512], in_=wvr[:, :, 0:512])
                nc.sync.dma_start(out=wv_sb[:, :, 512:1024], in_=wvr[:, :, 512:1024])

                # ---- K proj (own key half): per local jh quarter + CC ----
                for jh in range(2):
                    jhs = slice(jh * 512, (jh + 1) * 512)
                    if jh == 0:
                        # all of jh0 in 256-col chunks, chunk-outer: 7us of
                        # PE work per x2 quarter-chunk, so the PE never
                        # outruns the x2 DMA stream
                        for q in range(2):
                            qs = slice(q * 256, q * 256 + 256)
                            for et in range(ET):
                                pk = ps_proj.tile([128, 512], F32, tag="pp")
                                for d in range(DT):
                                    nc.tensor.matmul(
                                        pk[:, 0:256],
                                        wk_sb[:, d, et * 128 : (et + 1) * 128],
                                        x2_sb[:, d, qs],
                                        start=(d == 0),
                                        stop=(d == DT - 1),
                                    )
                                nc.scalar.activation(
                                    kt_own[:, et, qs],
                                    pk[:, 0:256],
                                    IDENT,
                                    bias=bk_sb[:, et : et + 1],
                                )
                    else:
                        for et in range(ET):
                            pk = ps_proj.tile([128, 512], F32, tag="pp")
                            for d in range(DT):
                                nc.tensor.matmul(
                                    pk,
                                    wk_sb[:, d, et * 128 : (et + 1) * 128],
                                    x2_sb[:, d, jhs],
                                    start=(d == 0),
                                    stop=(d == DT - 1),
                                )
                            nc.scalar.activation(
                                kt_own[:, et, jhs],
                                pk,
                                IDENT,
                                bias=bk_sb[:, et : et + 1],
                            )
                    nc.gpsimd.dma_start(out=kt_bin[jh], in_=kt_own[:, :, jhs])
                    nc.gpsimd.collective_compute(
                        "AllGather",
                        mybir.AluOpType.bypass,
                        replica_groups=PAIRS,
                        ins=[kt_bin[jh].opt()],
                        outs=[kt_bout[jh].opt()],
                    )
                    # slot r = rank r's local jh -> global keys 1024r + jhs
                    for r in range(2):
                        g0 = r * KH + jh * 512
                        nc.scalar.dma_start(
                            out=kt_sb[:, :, g0 : g0 + 512], in_=kt_bout[jh][r]
                        )

                # ---- Q proj: qt[e, i] ----
                for et in range(ET):
                    for ih in range(IH):
                        ihs = slice(ih * 512, (ih + 1) * 512)
                        pq = ps_proj.tile([128, 512], F32, tag="pp")
                        for d in range(DT):
                            nc.tensor.matmul(
                                pq,
                                wq_sb[:, d, et * 128 : (et + 1) * 128],
                                x1_sb[:, d, ihs],
                                start=(d == 0),
                                stop=(d == DT - 1),
                            )
                        nc.scalar.activation(
                            qt_sb[:, et, ihs], pq, IDENT, bias=bq_sb[:, et : et + 1]
                        )

                # ---- V proj (own key half): per local 4-jt quarter + CC ----
                for a in range(2):
                    for jt in range(a * 4, a * 4 + 4):
                        for eh in range(2):
                            pv = ps_proj.tile([128, 512], F32, tag="pp")
                            for d in range(DT):
                                nc.tensor.matmul(
                                    pv,
                                    x2_sb[:, d, jt * 128 : (jt + 1) * 128],
                                    wv_sb[:, d, eh * 512 : (eh + 1) * 512],
                                    start=(d == 0),
                                    stop=(d == DT - 1),
                                )
                            nc.scalar.activation(
                                v_own[:, jt, eh * 512 : (eh + 1) * 512], pv, IDENT
                            )
                    nc.gpsimd.dma_start(
                        out=v_bin[a], in_=v_own[:, a * 4 : a * 4 + 4, :]
                    )
                    nc.gpsimd.collective_compute(
                        "AllGather",
                        mybir.AluOpType.bypass,
                        replica_groups=PAIRS,
                        ins=[v_bin[a].opt()],
                        outs=[v_bout[a].opt()],
                    )
                    for r in range(2):
                        nc.scalar.dma_start(
                            out=v_sb[:, r * JT_L + a * 4 : r * JT_L + a * 4 + 4, :],
                            in_=v_bout[a][r],
                        )

            # ---- attention ----
            with (
                tc.tile_pool(name="ph2", bufs=1) as ph2,
                tc.tile_pool(name="stp", bufs=3) as stp,
                tc.tile_pool(name="rlp", bufs=2) as rlp,
            ):
                ex_sb = [
                    ph2.tile([128, JT_G, 512], BF16, tag=f"ex{ih}", name=f"ex{ih}")
                    for ih in range(IH)
                ]
                rlb_sb = ph2.tile([128, 512], F32, tag="rlbs")
                lps = []
                rlrs = []

                # scores, part-interleaved (S0-A, S1-A, S0-B, S1-B): the
                # B key-half CC gets a full extra scores pass of slack.
                # Softmax denominator matmuls lag one tile behind exp via a
                # per-ih pending chain, so the PE never waits.
                for ih in range(IH):
                    lps.append(ps_l.tile([1, 512], F32, tag="lp", name=f"lp{ih}"))
                pending = [None, None]
                cnt = [0, 0]
                for part in range(2):
                    seg = JTO[part * 8 : part * 8 + 8]
                    for ih in range(IH):
                        ihs = slice(ih * 512, (ih + 1) * 512)
                        ex = ex_sb[ih]
                        for jt in seg:
                            pst = ps_proj.tile([128, 512], F32, tag="pp")
                            for et in range(ET):
                                nc.tensor.matmul(
                                    pst,
                                    kt_sb[:, et, jt * 128 : (jt + 1) * 128],
                                    qt_sb[:, et, ihs],
                                    start=(et == 0),
                                    stop=(et == ET - 1),
                                )
                            nc.scalar.activation(
                                ex[:, jt, :], pst, EXP, scale=float(SCALE)
                            )
                            if pending[ih] is not None:
                                nc.tensor.matmul(
                                    lps[ih],
                                    onescb_sb[:, :],
                                    ex[:, pending[ih], :],
                                    start=(cnt[ih] == 0),
                                    stop=False,
                                )
                                cnt[ih] += 1
                            pending[ih] = jt
                        if part == 1:
                            nc.tensor.matmul(
                                lps[ih],
                                onescb_sb[:, :],
                                ex[:, pending[ih], :],
                                start=False,
                                stop=True,
                            )
                            rlr_sb = rlp.tile(
                                [1, 512], F32R, tag="rlr", name=f"rlr{ih}"
                            )
                            rlrs.append(rlr_sb)
                            with nc.allow_low_precision(
                                reason="f32r reciprocal feeds f32r broadcast"
                            ):
                                nc.vector.reciprocal(rlr_sb, lps[ih])

                # PV for both query halves
                for ih in range(IH):
                    ihs = slice(ih * 512, (ih + 1) * 512)
                    ex = ex_sb[ih]
                    rlb_ps = ps_rlb.tile([128, 512], F32, tag="rlb")
                    pots = []
                    for et in range(ET):
                        if ih == IH - 1 and et == ET - 1:
                            # last tile: two half-width pots so the closing
                            # DVE+DMA chain starts half a tile earlier
                            for q in range(2):
                                qs = slice(q * 256, q * 256 + 256)
                                oqs = slice(
                                    ih * 512 + q * 256, ih * 512 + q * 256 + 256
                                )
                                pot = ps_out.tile([128, 512], F32, tag="po")
                                for idx, jt in enumerate(JTO):
                                    nc.tensor.matmul(
                                        pot[:, 0:256],
                                        v_sb[:, jt, et * 128 : (et + 1) * 128],
                                        ex[:, jt, qs],
                                        start=(idx == 0),
                                        stop=(idx == JT_G - 1),
                                    )
                                st = stp.tile([128, 512], F32, tag="st")
                                nc.vector.tensor_mul(
                                    st[:, 0:256], pot[:, 0:256], rlb_sb[:, qs]
                                )
                                nc.vector.tensor_scalar_add(
                                    st[:, 0:256], st[:, 0:256], bvs_sb[:, et : et + 1]
                                )
                                nc.sync.dma_start(
                                    out=outT[et * 128 : (et + 1) * 128, oqs],
                                    in_=st[:, 0:256],
                                )
                            continue
                        pot = ps_out.tile([128, 512], F32, tag="po")
                        for idx, jt in enumerate(JTO):
                            nc.tensor.matmul(
                                pot,
                                v_sb[:, jt, et * 128 : (et + 1) * 128],
                                ex[:, jt, :],
                                start=(idx == 0),
                                stop=(idx == JT_G - 1),
                            )
                        if et < 2:
                            pots.append(pot)
                            if et == 1:
                                nc.tensor.matmul(
                                    rlb_ps,
                                    onesr_sb[:, :],
                                    rlrs[ih],
                                    start=True,
                                    stop=True,
                                )
                                nc.vector.tensor_copy(rlb_sb, rlb_ps)
                                for e2 in range(2):
                                    st = stp.tile([128, 512], F32, tag="st")
                                    nc.vector.tensor_mul(st, pots[e2], rlb_sb)
                                    nc.vector.tensor_scalar_add(
                                        st, st, bvs_sb[:, e2 : e2 + 1]
                                    )
                                    nc.sync.dma_start(
                                        out=outT[e2 * 128 : (e2 + 1) * 128, ihs],
                                        in_=st,
                                    )
                        else:
                            st = stp.tile([128, 512], F32, tag="st")
                            nc.vector.tensor_mul(st, pot, rlb_sb)
                            nc.vector.tensor_scalar_add(
                                st, st, bvs_sb[:, et : et + 1]
                            )
                            nc.sync.dma_start(
                                out=outT[et * 128 : (et + 1) * 128, ihs], in_=st
                            )

    nc.compile()
    return nc


def kernel(x1, x2, Wq, bq, Wk, bk, Wv, bv):
    global LAST_EXEC_NS, LAST_RESULTS

    x1 = np.asarray(x1, dtype=np.float32)
    x2 = np.asarray(x2, dtype=np.float32)
    Wq = np.asarray(Wq, dtype=np.float32)
    Wk = np.asarray(Wk, dtype=np.float32)
    Wv = np.asarray(Wv, dtype=np.float32)
    bq = np.asarray(bq, dtype=np.float32)
    bk = np.asarray(bk, dtype=np.float32)
    bv = np.asarray(bv, dtype=np.float32)

    if "nc" not in _CACHE:
        _CACHE["nc"] = _build()
    nc = _CACHE["nc"]

    WqT = np.ascontiguousarray(Wq.T).astype(NPBF16)
    WkT = np.ascontiguousarray(Wk.T).astype(NPBF16)
    WvT = np.ascontiguousarray(Wv.T).astype(NPBF16)
    bqs = np.ascontiguousarray(bq.reshape(8, 128).T)
    bks = np.ascontiguousarray(bk.reshape(8, 128).T)
    bvs = np.ascontiguousarray(bv.reshape(8, 128).T)
    onescb = np.ones((128, 1), dtype=NPBF16)
    onesr = np.ones((1, 128), dtype=np.float32)

    in_maps = []
    for c in range(N_CORES):
        b, h = divmod(c, 2)
        in_maps.append(
            {
                "x1hT": np.ascontiguousarray(
                    x1[b, h * QH : (h + 1) * QH, :].T
                ).astype(NPBF16),
                "x2hT": np.ascontiguousarray(
                    x2[b, h * KH : (h + 1) * KH, :].T
                ).astype(NPBF16),
                "WqT": WqT,
                "WkT": WkT,
                "WvT": WvT,
                "bqs": bqs,
                "bks": bks,
                "bvs": bvs,
                "onescb": onescb,
                "onesr": onesr,
            }
        )

    trace = os.environ.get("KERNEL_TRACE", "0") == "1" and _maybe_enable_trace()
    res = run_bass_kernel_spmd(nc, in_maps, list(range(N_CORES)), trace=trace)
    LAST_EXEC_NS = res.exec_time_ns
    LAST_RESULTS = res

    full = np.empty((B, SQ, D), dtype=np.float32)
    for c in range(N_CORES):
        b, h = divmod(c, 2)
        full[b, h * QH : (h + 1) * QH, :] = res.results[c]["outT"].T
    return full
